# revision 16
# baseline (speedup 1.0000x reference)
"""Causal self-attention on 8 Trainium2 NeuronCores.

Sharding: tensor-parallel over heads (16 heads -> 2 heads per core).
Each core computes q/k/v projections for its 2 heads, causal attention,
and a partial out-projection (rows of w_out for its heads). The host
sums the 8 partial [4096, 1024] outputs (the TP all-reduce).

v2 redesign vs the phased baseline (196.6us):
  - Single fully-interleaved emission schedule: the qkv-projection
    groups (PE-dense, full 128x128 array) are woven between attention
    iterations (PE at ~50% array use, Scalar-paced by exp), keeping
    TensorE dense the whole run and flattening the activity profile
    that triggered HAM 50%-throttle windows.
  - Normalize dance on-chip: softmax denominators come out of the PV
    ones-column, reciprocal'd on DVE straight out of PSUM, broadcast
    across the 128 head-dim partitions with a tiny [2,128]-indicator
    matmul on the PE (512 cols) instead of pack->DRAM->reciprocal->
    DRAM->broadcast-DMA round trips (which exposed a ~10us tail).
  - Causal masking via a precomputed [128,2,128] triangular bf16 mask
    multiplied on GpSimd (one 33k-elem op per diagonal tile) instead
    of gpsimd affine_select over the whole [128,1024] tile.
  - y staged through SBUF bf16 (DMA cannot read PSUM) with one DMA
    per 128-row chunk.
  - Software-pipelined attention inner loop: per k-chunk emission is
    S(k), exp(k), mask(k), PV(k-1), so the PE never sits behind the
    current exp; per-qc tails (normalize+out-proj) lag one iteration
    into the next q-chunk to hide the DVE reciprocal latency.
"""

import numpy as np
import ml_dtypes

import concourse.bacc as bacc
import concourse.mybir as mybir
from concourse.tile import TileContext
from concourse.bass_utils import run_bass_kernel_spmd

BF16 = mybir.dt.bfloat16
F32 = mybir.dt.float32
AF = mybir.ActivationFunctionType
ALU = mybir.AluOpType

NP_BF16 = np.dtype(ml_dtypes.bfloat16)

B, T, D_MODEL = 2, 2048, 1024
N_HEADS, HEAD_DIM = 16, 64
N_CORES = 8
HPC = N_HEADS // N_CORES          # heads per core (2)
DH = HEAD_DIM
HD = HPC * DH                     # 128 head-dims per core
SCALE = 1.0 / float(np.sqrt(DH))  # 0.125
QC = 512                          # q-chunk (free dim of S^T tiles)
KC = 128                          # k-chunk (partition dim of S^T tiles)


def weave(a, b):
    """Distribute b's units evenly between a's units (order preserved)."""
    if not a:
        return list(b)
    out = []
    na, nb, j = len(a), len(b), 0
    for i, u in enumerate(a):
        out.append(u)
        want = (i + 1) * nb // na
        while j < want:
            out.append(b[j])
            j += 1
    out.extend(b[j:])
    return out


def build_program(b=B, t=T, d=D_MODEL):
    rows = b * t
    dch = d // 128                # contraction chunks for the projections
    ng_w = 1024                   # x^T column-group width per proj group
    ngrp = rows // ng_w           # 4 groups
    rcpg = ng_w // 128            # row-chunks per group (8)
    nqc = t // QC                 # q-chunks per batch (4)
    rpq = QC // KC                # k-chunks per q-chunk (4)
    n_rchunk = rows // 128        # 32
    assert t % QC == 0 and d % 128 == 0 and rows % ng_w == 0

    nc = bacc.Bacc("TRN2", target_bir_lowering=False, debug=False,
                   num_devices=N_CORES)

    xT_d = nc.dram_tensor("xT", [d, rows], BF16, kind="ExternalInput")
    wqkv_d = nc.dram_tensor("wqkv", [d, 3 * HD], BF16, kind="ExternalInput")
    wo_d = nc.dram_tensor("wo", [HD, d], BF16, kind="ExternalInput")
    y_d = nc.dram_tensor("y", [rows, d], BF16, kind="ExternalOutput")

    with TileContext(nc) as tc:
        with tc.tile_pool(name="persist", bufs=1) as pp, \
             tc.tile_pool(name="xt", bufs=2 * dch) as pxt, \
             tc.tile_pool(name="pt", bufs=4) as ppt, \
             tc.tile_pool(name="pa", bufs=2) as pa, \
             tc.tile_pool(name="ysb", bufs=3) as py, \
             tc.tile_pool(name="psum", bufs=2, space="PSUM") as pps:
            wqkv = pp.tile([128, dch, 3 * HD], BF16)
            wo = pp.tile([HD, d], BF16)
            qT = pp.tile([HD, rows], BF16)
            kT = pp.tile([HD, rows], BF16)
            vT = pp.tile([HD, rows], BF16)
            v_aug = pp.tile([128, n_rchunk, HPC, DH + 1], BF16)
            v_nat = pp.tile([128, n_rchunk, HD], BF16)
            tri = pp.tile([128, HPC, KC], BF16)   # causal mask, q>=k keep
            # bc-matmul indicator: head h's recip row lives at partition
            # 32*h (engine partition offsets must be multiples of 32).
            # rsb rows 1..31 must stay zero (they enter the contraction),
            # so rsb is a persistent double buffer zeroed once at init.
            ind = pp.tile([64, HD], BF16)
            rsbs = [pp.tile([64, QC], BF16, name=f"rsb{i}")
                    for i in range(2)]

            nc.sync.dma_start(wqkv[:], wqkv_d.rearrange("(k p) m -> p k m", p=128))
            nc.sync.dma_start(wo[:], wo_d[:])
            nc.vector.memset(v_aug[:], 1.0)
            nc.gpsimd.memset(tri[:], 1.0)
            nc.gpsimd.affine_select(
                out=tri[:], in_=tri[:], compare_op=ALU.is_ge, fill=0.0,
                base=0, pattern=[[0, HPC], [1, KC]], channel_multiplier=-1)
            nc.vector.memset(ind[:], 0.0)
            nc.vector.memset(ind[0:1, 0:DH], 1.0)
            nc.vector.memset(ind[32:33, DH:HD], 1.0)
            nc.vector.memset(rsbs[0][:], 0.0)
            nc.vector.memset(rsbs[1][:], 0.0)

            xts = {}

            # ---------- projection units ----------
            def load_unit(g):
                def emit():
                    c0 = g * ng_w
                    tiles = []
                    for kc2 in range(dch):
                        xt = pxt.tile([128, ng_w], BF16, tag="xt", name="xt")
                        nc.sync.dma_start(
                            xt[:], xT_d[kc2 * 128:(kc2 + 1) * 128, c0:c0 + ng_w])
                        tiles.append(xt)
                    xts[g] = tiles
                return emit

            def chunk_unit(g, m, n2):
                def emit():
                    c0 = g * ng_w + n2 * 512
                    dst = (qT, kT, vT)[m]
                    ps = pps.tile([128, 512], F32, tag="sh", bufs=2, name="ps_proj")
                    for kc2 in range(dch):
                        nc.tensor.matmul(
                            ps[:],
                            wqkv[:, kc2, m * 128:(m + 1) * 128],
                            xts[g][kc2][:, n2 * 512:(n2 + 1) * 512],
                            start=(kc2 == 0), stop=(kc2 == dch - 1))
                    nc.vector.tensor_copy(dst[:, c0:c0 + 512], ps[:])
                return emit

            def trans_unit(g):
                def emit():
                    c0 = g * ng_w
                    r0 = g * rcpg
                    nc.sync.dma_start_transpose(
                        v_nat[:, r0:r0 + rcpg, :], vT[:, c0:c0 + ng_w])
                    for h in range(HPC):
                        nc.vector.tensor_copy(
                            v_aug[:, r0:r0 + rcpg, h, 0:DH],
                            v_nat[:, r0:r0 + rcpg, h * DH:(h + 1) * DH])
                return emit

            def proj_units(g):
                return ([chunk_unit(g, 2, n2) for n2 in range(2)]
                        + [trans_unit(g)]
                        + [chunk_unit(g, m, n2)
                           for m in range(2) for n2 in range(2)])

            # ---------- attention units ----------
            def emit_pv(bi, qc, kc, st):
                kpq = rpq * (qc + 1)
                pt, v0 = st['pts'].pop(kc)
                grc = (bi * t + kc * KC) // 128
                for h in range(HPC):
                    nc.tensor.matmul(
                        st['psO'][h][:, v0:],
                        v_aug[:, grc, h, :],
                        pt[:, h * QC + v0:(h + 1) * QC],
                        start=(kc == 0), stop=(kc == kpq - 1))

            def iter_unit(bi, qc, kc, st):
                def emit():
                    q0 = bi * t + qc * QC
                    kpq = rpq * (qc + 1)
                    if kc == 0:
                        st['psO'] = [
                            pps.tile([DH + 1, QC], F32, tag=f"psO{h}", bufs=1,
                                     name=f"psO{h}") for h in range(HPC)]
                    k0 = bi * t + kc * KC
                    v0 = max(0, (kc - rpq * qc) * KC)
                    ps_S = pps.tile([128, HPC * QC], F32, tag="S", bufs=2,
                                    name="ps_S")
                    for h in range(HPC):
                        nc.tensor.matmul(
                            ps_S[:, h * QC + v0:(h + 1) * QC],
                            kT[h * DH:(h + 1) * DH, k0:k0 + KC],
                            qT[h * DH:(h + 1) * DH, q0 + v0:q0 + QC],
                            start=True, stop=True)
                    pt = ppt.tile([128, HPC * QC], BF16, tag="pt", name="pt")
                    ps_S3 = ps_S.rearrange("p (h q) -> p h q", h=HPC)
                    pt3 = pt.rearrange("p (h q) -> p h q", h=HPC)
                    nc.scalar.activation(pt3[:, :, v0:], ps_S3[:, :, v0:],
                                         AF.Exp, scale=SCALE)
                    if kc >= rpq * qc:  # diagonal tile: triangular mask
                        nc.gpsimd.tensor_mul(
                            pt3[:, :, v0:v0 + KC], pt3[:, :, v0:v0 + KC],
                            tri[:])
                    st['pts'][kc] = (pt, v0)
                    if kc > 0:
                        emit_pv(bi, qc, kc - 1, st)
                    if kc == kpq - 1:
                        emit_pv(bi, qc, kc, st)
                return emit

            def tail_early_unit(bi, qc, st):
                def emit():
                    aT = pa.tile([HD, QC], BF16, tag="aT", bufs=2, name="aT")
                    rsb = rsbs[(bi * nqc + qc) % 2]
                    for h in range(HPC):
                        nc.vector.tensor_copy(
                            aT[h * DH:(h + 1) * DH, :], st['psO'][h][0:DH, :])
                        with nc.allow_low_precision(
                                reason="softmax denominators are O(100) and "
                                       "the output feeds a bf16 matmul"):
                            nc.vector.reciprocal(
                                rsb[32 * h:32 * h + 1, :],
                                st['psO'][h][DH:DH + 1, :])
                    st['aT'] = aT
                    st['rsb'] = rsb
                return emit

            def tail_bc_unit(bi, qc, st):
                def emit():
                    ps_bc = pps.tile([HD, QC], F32, tag="sh", bufs=2,
                                     name="ps_bc")
                    nc.tensor.matmul(ps_bc[:], ind[:], st['rsb'][:],
                                     start=True, stop=True)
                    aTn = pa.tile([HD, QC], BF16, tag="aTn", bufs=2,
                                  name="aTn")
                    nc.vector.tensor_mul(aTn[:], st['aT'][:], ps_bc[:])
                    st['aTn'] = aTn
                return emit

            def outproj_unit(bi, qc, rc4, st):
                def emit():
                    rc = (bi * t + qc * QC) // 128 + rc4
                    ysb = py.tile([128, d], BF16, tag="ysb", name="ysb")
                    for n2 in range(2):
                        ps_y = pps.tile([128, 512], F32, tag="sh", bufs=2,
                                        name="ps_y")
                        nc.tensor.matmul(
                            ps_y[:],
                            st['aTn'][:, rc4 * 128:(rc4 + 1) * 128],
                            wo[:, n2 * 512:(n2 + 1) * 512],
                            start=True, stop=True)
                        nc.vector.tensor_copy(
                            ysb[:, n2 * 512:(n2 + 1) * 512], ps_y[:])
                    nc.sync.dma_start(y_d[rc * 128:(rc + 1) * 128, :], ysb[:])
                return emit

            # ---------- build the global schedule ----------
            attn_segs = [[], [], [], []]
            pending_early = None
            pending_late = []
            for bi in range(b):
                for qc in range(nqc):
                    seg = attn_segs[2 * bi + qc // 2]
                    st = {'pts': {}}
                    kpq = rpq * (qc + 1)
                    units = [iter_unit(bi, qc, kc, st) for kc in range(kpq)]
                    pre = [pending_early] if pending_early else []
                    seg += pre + units[:2] + weave(units[2:], pending_late)
                    pending_early = tail_early_unit(bi, qc, st)
                    pending_late = ([tail_bc_unit(bi, qc, st)]
                                    + [outproj_unit(bi, qc, r, st)
                                       for r in range(4)])
            attn_segs[3] += [pending_early] + pending_late

            sched = ([load_unit(0), load_unit(1)] + proj_units(0)
                     + weave(attn_segs[0], [load_unit(2)] + proj_units(1))
                     + weave(attn_segs[1], [load_unit(3)] + proj_units(2))
                     + weave(attn_segs[2], proj_units(3))
                     + attn_segs[3])
            for u in sched:
                u()

    nc.compile()
    return nc


def make_in_maps(x, w_qkv, w_out, b=B, t=T, d=D_MODEL):
    rows = b * t
    xr = np.asarray(x, dtype=np.float32).reshape(rows, d)
    xT = np.ascontiguousarray(xr.T).astype(NP_BF16)
    wq = np.asarray(w_qkv[:, 0:d]).reshape(d, N_HEADS, DH)
    wk = np.asarray(w_qkv[:, d:2 * d]).reshape(d, N_HEADS, DH)
    wvf = np.asarray(w_qkv[:, 2 * d:3 * d]).reshape(d, N_HEADS, DH)
    in_maps = []
    for c in range(N_CORES):
        h0, h1 = HPC * c, HPC * c + HPC
        wqkv_c = np.concatenate(
            [wq[:, h0:h1].reshape(d, HD), wk[:, h0:h1].reshape(d, HD),
             wvf[:, h0:h1].reshape(d, HD)], axis=1).astype(NP_BF16)
        wo_c = np.ascontiguousarray(w_out[h0 * DH:h1 * DH, :]).astype(NP_BF16)
        in_maps.append({"xT": xT, "wqkv": wqkv_c, "wo": wo_c})
    return in_maps


_PROGRAM_CACHE = {}


def _get_program():
    if "nc" not in _PROGRAM_CACHE:
        _PROGRAM_CACHE["nc"] = build_program()
    return _PROGRAM_CACHE["nc"]


def run(x, w_qkv, w_out, trace=False, tmpdir=None):
    nc = _get_program()
    in_maps = make_in_maps(x, w_qkv, w_out)
    res = run_bass_kernel_spmd(nc, in_maps, list(range(N_CORES)), trace=trace,
                               tmpdir=tmpdir)
    parts = np.stack([np.asarray(res.results[c]["y"], dtype=np.float32)
                      for c in range(N_CORES)])
    y = parts.sum(axis=0).reshape(B, T, D_MODEL)
    return y, res


def kernel(x, w_qkv, w_out):
    y, _ = run(x, w_qkv, w_out)
    return y


# revision 21
# speedup vs baseline: 1.0909x; 1.0909x over previous
"""Causal self-attention on 8 Trainium2 NeuronCores.

Sharding: tensor-parallel over heads (16 heads -> 2 heads per core).
Each core computes q/k/v projections for its 2 heads, causal attention,
and a partial out-projection (rows of w_out for its heads). The host
sums the 8 partial [4096, 1024] outputs (the TP all-reduce).

v2 redesign vs the phased baseline (196.6us):
  - Single fully-interleaved emission schedule: the qkv-projection
    groups (PE-dense, full 128x128 array) are woven between attention
    iterations (PE at ~50% array use, Scalar-paced by exp), keeping
    TensorE dense the whole run and flattening the activity profile
    that triggered HAM 50%-throttle windows.
  - Normalize dance on-chip: softmax denominators come out of the PV
    ones-column, reciprocal'd on DVE straight out of PSUM, broadcast
    across the 128 head-dim partitions with a tiny [2,128]-indicator
    matmul on the PE (512 cols) instead of pack->DRAM->reciprocal->
    DRAM->broadcast-DMA round trips (which exposed a ~10us tail).
  - Causal masking via a precomputed [128,2,128] triangular bf16 mask
    multiplied on GpSimd (one 33k-elem op per diagonal tile) instead
    of gpsimd affine_select over the whole [128,1024] tile.
  - y staged through SBUF bf16 (DMA cannot read PSUM) with one DMA
    per 128-row chunk.
  - Software-pipelined attention inner loop: per k-chunk emission is
    S(k), exp(k), mask(k), PV(k-1), so the PE never sits behind the
    current exp; per-qc tails (normalize+out-proj) lag one iteration
    into the next q-chunk to hide the DVE reciprocal latency.
"""

import numpy as np
import ml_dtypes

import concourse.bacc as bacc
import concourse.mybir as mybir
from concourse.tile import TileContext
from concourse.bass_utils import run_bass_kernel_spmd

BF16 = mybir.dt.bfloat16
F32 = mybir.dt.float32
AF = mybir.ActivationFunctionType
ALU = mybir.AluOpType

NP_BF16 = np.dtype(ml_dtypes.bfloat16)

B, T, D_MODEL = 2, 2048, 1024
N_HEADS, HEAD_DIM = 16, 64
N_CORES = 8
HPC = N_HEADS // N_CORES          # heads per core (2)
DH = HEAD_DIM
HD = HPC * DH                     # 128 head-dims per core
SCALE = 1.0 / float(np.sqrt(DH))  # 0.125
QC = 512                          # q-chunk (free dim of S^T tiles)
KC = 128                          # k-chunk (partition dim of S^T tiles)


def weave(a, b):
    """Distribute b's units evenly between a's units (order preserved)."""
    if not a:
        return list(b)
    out = []
    na, nb, j = len(a), len(b), 0
    for i, u in enumerate(a):
        out.append(u)
        want = (i + 1) * nb // na
        while j < want:
            out.append(b[j])
            j += 1
    out.extend(b[j:])
    return out


def build_program(b=B, t=T, d=D_MODEL):
    rows = b * t
    dch = d // 128                # contraction chunks for the projections
    ng_w = 1024                   # x^T column-group width per proj group
    ngrp = rows // ng_w           # 4 groups
    rcpg = ng_w // 128            # row-chunks per group (8)
    nqc = t // QC                 # q-chunks per batch (4)
    rpq = QC // KC                # k-chunks per q-chunk (4)
    n_rchunk = rows // 128        # 32
    assert t % QC == 0 and d % 128 == 0 and rows % ng_w == 0

    nc = bacc.Bacc("TRN2", target_bir_lowering=False, debug=False,
                   num_devices=N_CORES)

    xT_d = nc.dram_tensor("xT", [d, rows], BF16, kind="ExternalInput")
    wqkv_d = nc.dram_tensor("wqkv", [d, 3 * HD], BF16, kind="ExternalInput")
    wo_d = nc.dram_tensor("wo", [HD, d], BF16, kind="ExternalInput")
    y_d = nc.dram_tensor("y", [rows, d], BF16, kind="ExternalOutput")

    with TileContext(nc) as tc:
        with tc.tile_pool(name="persist", bufs=1) as pp, \
             tc.tile_pool(name="xt", bufs=2 * dch) as pxt, \
             tc.tile_pool(name="pt", bufs=4) as ppt, \
             tc.tile_pool(name="pa", bufs=2) as pa, \
             tc.tile_pool(name="ysb", bufs=3) as py, \
             tc.tile_pool(name="dramtmp", bufs=4, space="DRAM") as pd, \
             tc.tile_pool(name="psum", bufs=2, space="PSUM") as pps:
            wqkv = pp.tile([128, dch, 3 * HD], BF16)
            wo = pp.tile([HD, d], BF16)
            qT = pp.tile([HD, rows], BF16)
            kT = pp.tile([HD, rows], BF16)
            vT = pp.tile([HD, rows], BF16)
            v_aug = pp.tile([128, n_rchunk, HPC, DH + 1], BF16)
            v_nat = pp.tile([128, n_rchunk, HD], BF16)
            tri = pp.tile([128, HPC, KC], BF16)   # causal mask, q>=k keep
            # bc-matmul indicator: head h's recip row lives at partition
            # 32*h (engine partition offsets must be multiples of 32).
            # rsb rows 1..31 must stay zero (they enter the contraction),
            # so rsb is a persistent double buffer zeroed once at init.
            ind = pp.tile([64, HD], BF16)
            rsbs = [pp.tile([64, QC], BF16, name=f"rsb{i}")
                    for i in range(2)]

            nc.sync.dma_start(wqkv[:], wqkv_d.rearrange("(k p) m -> p k m", p=128))
            nc.sync.dma_start(wo[:], wo_d[:])
            nc.vector.memset(v_aug[:], 1.0)
            nc.gpsimd.memset(tri[:], 1.0)
            nc.gpsimd.affine_select(
                out=tri[:], in_=tri[:], compare_op=ALU.is_ge, fill=0.0,
                base=0, pattern=[[0, HPC], [1, KC]], channel_multiplier=-1)
            nc.vector.memset(ind[:], 0.0)
            nc.vector.memset(ind[0:1, 0:DH], 1.0)
            nc.vector.memset(ind[32:33, DH:HD], 1.0)
            nc.vector.memset(rsbs[0][:], 0.0)
            nc.vector.memset(rsbs[1][:], 0.0)

            xts = {}

            # ---------- projection units ----------
            def load_unit(g):
                def emit():
                    c0 = g * ng_w
                    tiles = []
                    for kc2 in range(dch):
                        xt = pxt.tile([128, ng_w], BF16, tag="xt", name="xt")
                        nc.sync.dma_start(
                            xt[:], xT_d[kc2 * 128:(kc2 + 1) * 128, c0:c0 + ng_w])
                        tiles.append(xt)
                    xts[g] = tiles
                return emit

            def chunk_unit(g, m, n2):
                def emit():
                    c0 = g * ng_w + n2 * 512
                    dst = (qT, kT, vT)[m]
                    ps = pps.tile([128, 512], F32, tag="sh", bufs=2, name="ps_proj")
                    for kc2 in range(dch):
                        nc.tensor.matmul(
                            ps[:],
                            wqkv[:, kc2, m * 128:(m + 1) * 128],
                            xts[g][kc2][:, n2 * 512:(n2 + 1) * 512],
                            start=(kc2 == 0), stop=(kc2 == dch - 1))
                    nc.vector.tensor_copy(dst[:, c0:c0 + 512], ps[:])
                return emit

            def trans_unit(g):
                def emit():
                    c0 = g * ng_w
                    r0 = g * rcpg
                    nc.sync.dma_start_transpose(
                        v_nat[:, r0:r0 + rcpg, :], vT[:, c0:c0 + ng_w])
                    for h in range(HPC):
                        nc.vector.tensor_copy(
                            v_aug[:, r0:r0 + rcpg, h, 0:DH],
                            v_nat[:, r0:r0 + rcpg, h * DH:(h + 1) * DH])
                return emit

            def proj_units(g):
                return ([chunk_unit(g, 2, n2) for n2 in range(2)]
                        + [trans_unit(g)]
                        + [chunk_unit(g, m, n2)
                           for m in range(2) for n2 in range(2)])

            # ---------- attention units ----------
            def emit_pv(bi, qc, kc, st):
                kpq = rpq * (qc + 1)
                pt, v0 = st['pts'].pop(kc)
                grc = (bi * t + kc * KC) // 128
                for h in range(HPC):
                    nc.tensor.matmul(
                        st['psO'][h][:, v0:],
                        v_aug[:, grc, h, :],
                        pt[:, h * QC + v0:(h + 1) * QC],
                        start=(kc == 0), stop=(kc == kpq - 1))

            def iter_unit(bi, qc, kc, st):
                def emit():
                    q0 = bi * t + qc * QC
                    kpq = rpq * (qc + 1)
                    if kc == 0:
                        st['psO'] = [
                            pps.tile([DH + 1, QC], F32, tag=f"psO{h}", bufs=1,
                                     name=f"psO{h}") for h in range(HPC)]
                    k0 = bi * t + kc * KC
                    v0 = max(0, (kc - rpq * qc) * KC)
                    ps_S = pps.tile([128, HPC * QC], F32, tag="S", bufs=2,
                                    name="ps_S")
                    for h in range(HPC):
                        nc.tensor.matmul(
                            ps_S[:, h * QC + v0:(h + 1) * QC],
                            kT[h * DH:(h + 1) * DH, k0:k0 + KC],
                            qT[h * DH:(h + 1) * DH, q0 + v0:q0 + QC],
                            start=True, stop=True)
                    pt = ppt.tile([128, HPC * QC], BF16, tag="pt", name="pt")
                    ps_S3 = ps_S.rearrange("p (h q) -> p h q", h=HPC)
                    pt3 = pt.rearrange("p (h q) -> p h q", h=HPC)
                    nc.scalar.activation(pt3[:, :, v0:], ps_S3[:, :, v0:],
                                         AF.Exp, scale=SCALE)
                    if kc >= rpq * qc:  # diagonal tile: triangular mask
                        nc.gpsimd.tensor_mul(
                            pt3[:, :, v0:v0 + KC], pt3[:, :, v0:v0 + KC],
                            tri[:])
                    st['pts'][kc] = (pt, v0)
                    if kc > 0:
                        emit_pv(bi, qc, kc - 1, st)
                    if kc == kpq - 1:
                        emit_pv(bi, qc, kc, st)
                return emit

            def tail_early_unit(bi, qc, st):
                def emit():
                    # DVE reciprocal cost ~6.5ns * free_size (partitions are
                    # free), so bounce the 2x512 sums through a [64,16]
                    # layout via SBUF->SBUF DMAs to make it ~100ns.
                    aT = pa.tile([HD, QC], BF16, tag="aT", bufs=2, name="aT")
                    srow = pa.tile([64, QC], F32, tag="srow", bufs=2,
                                   name="srow")
                    rp = pa.tile([64, 16], F32, tag="rp", bufs=2, name="rp")
                    rpb = pa.tile([64, 16], BF16, tag="rpb", bufs=2,
                                  name="rpb")
                    rsb = rsbs[(bi * nqc + qc) % 2]
                    for h in range(HPC):
                        nc.vector.tensor_copy(
                            aT[h * DH:(h + 1) * DH, :], st['psO'][h][0:DH, :])
                        nc.vector.tensor_copy(
                            srow[32 * h:32 * h + 1, :],
                            st['psO'][h][DH:DH + 1, :])
                    # partition fan-out/fan-in needs a DRAM bounce: a plain
                    # SBUF->SBUF DMA cannot restructure partitions.
                    s_d = pd.tile([HPC, QC], F32, tag="s_d", bufs=2,
                                  name="s_d")
                    r_d = pd.tile([64, 16], BF16, tag="r_d", bufs=2,
                                  name="r_d")
                    nc.sync.dma_start(
                        s_d[:],
                        srow.rearrange("(h r) q -> h r q", h=HPC)[:, 0, :])
                    nc.sync.dma_start(
                        rp[:], s_d.rearrange("h (a f) -> (h a) f", f=16))
                    with nc.allow_low_precision(
                            reason="softmax denominators are O(100) and "
                                   "the output feeds a bf16 matmul"):
                        nc.vector.reciprocal(rpb[:], rp[:])
                    nc.sync.dma_start(r_d[:], rpb[:])
                    nc.sync.dma_start(
                        rsb.rearrange("(h r) q -> h r q", h=HPC)[:, 0, :],
                        r_d.rearrange("(h a) f -> h (a f)", h=HPC))
                    st['aT'] = aT
                    st['rsb'] = rsb
                return emit

            def tail_bc_unit(bi, qc, st):
                def emit():
                    ps_bc = pps.tile([HD, QC], F32, tag="sh", bufs=2,
                                     name="ps_bc")
                    nc.tensor.matmul(ps_bc[:], ind[:], st['rsb'][:],
                                     start=True, stop=True)
                    aTn = pa.tile([HD, QC], BF16, tag="aTn", bufs=2,
                                  name="aTn")
                    nc.vector.tensor_mul(aTn[:], st['aT'][:], ps_bc[:])
                    st['aTn'] = aTn
                return emit

            def outproj_unit(bi, qc, rc4, st):
                def emit():
                    rc = (bi * t + qc * QC) // 128 + rc4
                    ysb = py.tile([128, d], BF16, tag="ysb", name="ysb")
                    for n2 in range(2):
                        ps_y = pps.tile([128, 512], F32, tag="sh", bufs=2,
                                        name="ps_y")
                        nc.tensor.matmul(
                            ps_y[:],
                            st['aTn'][:, rc4 * 128:(rc4 + 1) * 128],
                            wo[:, n2 * 512:(n2 + 1) * 512],
                            start=True, stop=True)
                        nc.vector.tensor_copy(
                            ysb[:, n2 * 512:(n2 + 1) * 512], ps_y[:])
                    nc.sync.dma_start(y_d[rc * 128:(rc + 1) * 128, :], ysb[:])
                return emit

            # ---------- build the global schedule ----------
            attn_segs = [[], [], [], []]
            pending_early = None
            pending_late = []
            for bi in range(b):
                for qc in range(nqc):
                    seg = attn_segs[2 * bi + qc // 2]
                    st = {'pts': {}}
                    kpq = rpq * (qc + 1)
                    units = [iter_unit(bi, qc, kc, st) for kc in range(kpq)]
                    pre = [pending_early] if pending_early else []
                    seg += pre + units[:2] + weave(units[2:], pending_late)
                    pending_early = tail_early_unit(bi, qc, st)
                    pending_late = ([tail_bc_unit(bi, qc, st)]
                                    + [outproj_unit(bi, qc, r, st)
                                       for r in range(4)])
            attn_segs[3] += [pending_early] + pending_late

            sched = ([load_unit(0), load_unit(1)] + proj_units(0)
                     + weave(attn_segs[0], [load_unit(2)] + proj_units(1))
                     + weave(attn_segs[1], [load_unit(3)] + proj_units(2))
                     + weave(attn_segs[2], proj_units(3))
                     + attn_segs[3])
            for u in sched:
                u()

    nc.compile()
    return nc


def make_in_maps(x, w_qkv, w_out, b=B, t=T, d=D_MODEL):
    rows = b * t
    xr = np.asarray(x, dtype=np.float32).reshape(rows, d)
    xT = np.ascontiguousarray(xr.T).astype(NP_BF16)
    wq = np.asarray(w_qkv[:, 0:d]).reshape(d, N_HEADS, DH)
    wk = np.asarray(w_qkv[:, d:2 * d]).reshape(d, N_HEADS, DH)
    wvf = np.asarray(w_qkv[:, 2 * d:3 * d]).reshape(d, N_HEADS, DH)
    in_maps = []
    for c in range(N_CORES):
        h0, h1 = HPC * c, HPC * c + HPC
        wqkv_c = np.concatenate(
            [wq[:, h0:h1].reshape(d, HD), wk[:, h0:h1].reshape(d, HD),
             wvf[:, h0:h1].reshape(d, HD)], axis=1).astype(NP_BF16)
        wo_c = np.ascontiguousarray(w_out[h0 * DH:h1 * DH, :]).astype(NP_BF16)
        in_maps.append({"xT": xT, "wqkv": wqkv_c, "wo": wo_c})
    return in_maps


_PROGRAM_CACHE = {}


def _get_program():
    if "nc" not in _PROGRAM_CACHE:
        _PROGRAM_CACHE["nc"] = build_program()
    return _PROGRAM_CACHE["nc"]


def run(x, w_qkv, w_out, trace=False, tmpdir=None):
    nc = _get_program()
    in_maps = make_in_maps(x, w_qkv, w_out)
    res = run_bass_kernel_spmd(nc, in_maps, list(range(N_CORES)), trace=trace,
                               tmpdir=tmpdir)
    parts = np.stack([np.asarray(res.results[c]["y"], dtype=np.float32)
                      for c in range(N_CORES)])
    y = parts.sum(axis=0).reshape(B, T, D_MODEL)
    return y, res


def kernel(x, w_qkv, w_out):
    y, _ = run(x, w_qkv, w_out)
    return y


# revision 27
# speedup vs baseline: 1.2353x; 1.1324x over previous
"""Causal self-attention on 8 Trainium2 NeuronCores.

Sharding: tensor-parallel over heads (16 heads -> 2 heads per core).
Each core computes q/k/v projections for its 2 heads, causal attention,
and a partial out-projection (rows of w_out for its heads). The host
sums the 8 partial [4096, 1024] outputs (the TP all-reduce).

v2 redesign vs the phased baseline (196.6us):
  - Single fully-interleaved emission schedule: the qkv-projection
    groups (PE-dense, full 128x128 array) are woven between attention
    iterations (PE at ~50% array use, Scalar-paced by exp), keeping
    TensorE dense the whole run and flattening the activity profile
    that triggered HAM 50%-throttle windows.
  - Normalize dance on-chip: softmax denominators come out of the PV
    ones-column, reciprocal'd on DVE straight out of PSUM, broadcast
    across the 128 head-dim partitions with a tiny [2,128]-indicator
    matmul on the PE (512 cols) instead of pack->DRAM->reciprocal->
    DRAM->broadcast-DMA round trips (which exposed a ~10us tail).
  - Causal masking via a precomputed [128,2,128] triangular bf16 mask
    multiplied on GpSimd (one 33k-elem op per diagonal tile) instead
    of gpsimd affine_select over the whole [128,1024] tile.
  - y staged through SBUF bf16 (DMA cannot read PSUM) with one DMA
    per 128-row chunk.
  - Software-pipelined attention inner loop: per k-chunk emission is
    S(k), exp(k), mask(k), PV(k-1), so the PE never sits behind the
    current exp; per-qc tails (normalize+out-proj) lag one iteration
    into the next q-chunk to hide the DVE reciprocal latency.
"""

import numpy as np
import ml_dtypes

import concourse.bacc as bacc
import concourse.mybir as mybir
from concourse.tile import TileContext
from concourse.bass_utils import run_bass_kernel_spmd

BF16 = mybir.dt.bfloat16
F32 = mybir.dt.float32
AF = mybir.ActivationFunctionType
ALU = mybir.AluOpType

NP_BF16 = np.dtype(ml_dtypes.bfloat16)

B, T, D_MODEL = 2, 2048, 1024
N_HEADS, HEAD_DIM = 16, 64
N_CORES = 8
HPC = N_HEADS // N_CORES          # heads per core (2)
DH = HEAD_DIM
HD = HPC * DH                     # 128 head-dims per core
SCALE = 1.0 / float(np.sqrt(DH))  # 0.125
QC = 512                          # q-chunk (free dim of S^T tiles)
KC = 128                          # k-chunk (partition dim of S^T tiles)


def weave(a, b):
    """Distribute b's units evenly between a's units (order preserved)."""
    if not a:
        return list(b)
    out = []
    na, nb, j = len(a), len(b), 0
    for i, u in enumerate(a):
        out.append(u)
        want = (i + 1) * nb // na
        while j < want:
            out.append(b[j])
            j += 1
    out.extend(b[j:])
    return out


def build_program(b=B, t=T, d=D_MODEL):
    rows = b * t
    dch = d // 128                # contraction chunks for the projections
    ng_w = 1024                   # x^T column-group width per proj group
    ngrp = rows // ng_w           # 4 groups
    rcpg = ng_w // 128            # row-chunks per group (8)
    nqc = t // QC                 # q-chunks per batch (4)
    rpq = QC // KC                # k-chunks per q-chunk (4)
    n_rchunk = rows // 128        # 32
    assert t % QC == 0 and d % 128 == 0 and rows % ng_w == 0

    nc = bacc.Bacc("TRN2", target_bir_lowering=False, debug=False,
                   num_devices=N_CORES)

    xT_d = nc.dram_tensor("xT", [d, rows], BF16, kind="ExternalInput")
    wqkv_d = nc.dram_tensor("wqkv", [d, 3 * HD], BF16, kind="ExternalInput")
    wo_d = nc.dram_tensor("wo", [HD, d], BF16, kind="ExternalInput")
    y_d = nc.dram_tensor("y", [rows, d], BF16, kind="ExternalOutput")

    with TileContext(nc) as tc:
        with tc.tile_pool(name="persist", bufs=1) as pp, \
             tc.tile_pool(name="xt", bufs=2 * dch) as pxt, \
             tc.tile_pool(name="pt", bufs=4) as ppt, \
             tc.tile_pool(name="pa", bufs=2) as pa, \
             tc.tile_pool(name="ysb", bufs=3) as py, \
             tc.tile_pool(name="psum", bufs=2, space="PSUM") as pps:
            wqkv = pp.tile([128, dch, 3 * HD], BF16)
            wo = pp.tile([HD, d], BF16)
            qT = pp.tile([HD, rows], BF16)
            kT = pp.tile([HD, rows], BF16)
            vT = pp.tile([HD, rows], BF16)
            v_aug = pp.tile([128, n_rchunk, HPC, DH + 1], BF16)
            v_nat = pp.tile([128, n_rchunk, HD], BF16)
            tri = pp.tile([128, HPC, KC], BF16)   # causal mask, q>=k keep
            # bc-matmul indicator: head h's sums row lives at partition
            # 32*h (engine partition offsets must be multiples of 32).
            # srow rows 1..31/33..63 must stay zero (they enter the
            # contraction), so srow is a persistent pair zeroed at init.
            ind = pp.tile([64, HD], BF16)
            srows = [pp.tile([64, QC], BF16, name=f"srow{i}")
                     for i in range(2)]

            nc.scalar.dma_start(wqkv[:],
                                wqkv_d.rearrange("(k p) m -> p k m", p=128))
            nc.scalar.dma_start(wo[:], wo_d[:])
            nc.vector.memset(v_aug[:], 1.0)
            nc.gpsimd.memset(tri[:], 1.0)
            nc.gpsimd.affine_select(
                out=tri[:], in_=tri[:], compare_op=ALU.is_ge, fill=0.0,
                base=0, pattern=[[0, HPC], [1, KC]], channel_multiplier=-1)
            nc.vector.memset(ind[:], 0.0)
            nc.vector.memset(ind[0:1, 0:DH], 1.0)
            nc.vector.memset(ind[32:33, DH:HD], 1.0)
            nc.vector.memset(srows[0][:], 0.0)
            nc.vector.memset(srows[1][:], 0.0)

            xts = {}

            # ---------- projection units ----------
            def load_unit(g):
                def emit():
                    c0 = g * ng_w
                    tiles = []
                    for kc2 in range(dch):
                        xt = pxt.tile([128, ng_w], BF16, tag="xt", name="xt")
                        nc.sync.dma_start(
                            xt[:], xT_d[kc2 * 128:(kc2 + 1) * 128, c0:c0 + ng_w])
                        tiles.append(xt)
                    xts[g] = tiles
                return emit

            def chunk_unit(g, m, n2):
                def emit():
                    c0 = g * ng_w + n2 * 512
                    dst = (qT, kT, vT)[m]
                    ps = pps.tile([128, 512], F32, tag="sh", bufs=2, name="ps_proj")
                    for kc2 in range(dch):
                        nc.tensor.matmul(
                            ps[:],
                            wqkv[:, kc2, m * 128:(m + 1) * 128],
                            xts[g][kc2][:, n2 * 512:(n2 + 1) * 512],
                            start=(kc2 == 0), stop=(kc2 == dch - 1))
                    nc.vector.tensor_copy(dst[:, c0:c0 + 512], ps[:])
                return emit

            def trans_unit(g):
                def emit():
                    c0 = g * ng_w
                    r0 = g * rcpg
                    nc.scalar.dma_start_transpose(
                        v_nat[:, r0:r0 + rcpg, :], vT[:, c0:c0 + ng_w])
                    for h in range(HPC):
                        nc.vector.tensor_copy(
                            v_aug[:, r0:r0 + rcpg, h, 0:DH],
                            v_nat[:, r0:r0 + rcpg, h * DH:(h + 1) * DH])
                return emit

            def proj_units(g):
                return ([chunk_unit(g, 2, n2) for n2 in range(2)]
                        + [trans_unit(g)]
                        + [chunk_unit(g, m, n2)
                           for m in range(2) for n2 in range(2)])

            # ---------- attention units ----------
            def emit_pv(bi, qc, kc, st):
                kpq = rpq * (qc + 1)
                pt, v0 = st['pts'].pop(kc)
                grc = (bi * t + kc * KC) // 128
                for h in range(HPC):
                    nc.tensor.matmul(
                        st['psO'][h][:, v0:],
                        v_aug[:, grc, h, :],
                        pt[:, h * QC + v0:(h + 1) * QC],
                        start=(kc == 0), stop=(kc == kpq - 1))

            def iter_unit(bi, qc, kc, st):
                def emit():
                    q0 = bi * t + qc * QC
                    kpq = rpq * (qc + 1)
                    if kc == 0:
                        st['psO'] = [
                            pps.tile([DH + 1, QC], F32, tag=f"psO{h}", bufs=1,
                                     name=f"psO{h}") for h in range(HPC)]
                    k0 = bi * t + kc * KC
                    v0 = max(0, (kc - rpq * qc) * KC)
                    ps_S = pps.tile([128, HPC * QC], F32, tag="S", bufs=2,
                                    name="ps_S")
                    for h in range(HPC):
                        nc.tensor.matmul(
                            ps_S[:, h * QC + v0:(h + 1) * QC],
                            kT[h * DH:(h + 1) * DH, k0:k0 + KC],
                            qT[h * DH:(h + 1) * DH, q0 + v0:q0 + QC],
                            start=True, stop=True)
                    pt = ppt.tile([128, HPC * QC], BF16, tag="pt", name="pt")
                    ps_S3 = ps_S.rearrange("p (h q) -> p h q", h=HPC)
                    pt3 = pt.rearrange("p (h q) -> p h q", h=HPC)
                    nc.scalar.activation(pt3[:, :, v0:], ps_S3[:, :, v0:],
                                         AF.Exp, scale=SCALE)
                    if kc >= rpq * qc:  # diagonal tile: triangular mask
                        nc.gpsimd.tensor_mul(
                            pt3[:, :, v0:v0 + KC], pt3[:, :, v0:v0 + KC],
                            tri[:])
                    st['pts'][kc] = (pt, v0)
                    if kc > 0:
                        emit_pv(bi, qc, kc - 1, st)
                    if kc == kpq - 1:
                        emit_pv(bi, qc, kc, st)
                return emit

            def tail_early_unit(bi, qc, st):
                def emit():
                    # drain U^T and the sums row; srow is already the
                    # row-layout the broadcast matmul wants, so no
                    # reciprocal dance: broadcast raw sums and divide.
                    aT = pa.tile([HD, QC], BF16, tag="aT", bufs=2, name="aT")
                    srow = srows[(bi * nqc + qc) % 2]
                    for h in range(HPC):
                        nc.vector.tensor_copy(
                            aT[h * DH:(h + 1) * DH, :], st['psO'][h][0:DH, :])
                        with nc.allow_low_precision(
                                reason="softmax denominators are O(100), "
                                       "bf16 matches the old recip path"):
                            nc.vector.tensor_copy(
                                srow[32 * h:32 * h + 1, :],
                                st['psO'][h][DH:DH + 1, :])
                    st['aT'] = aT
                    st['srow'] = srow
                return emit

            def tail_bc_unit(bi, qc, st):
                def emit():
                    # broadcast raw sums over the 128 head-dim partitions,
                    # then one fast approximate reciprocal (18 bits, way
                    # beyond bf16) and a multiply.
                    ps_bc = pps.tile([HD, QC], F32, tag="sh", bufs=2,
                                     name="ps_bc")
                    nc.tensor.matmul(ps_bc[:], ind[:], st['srow'][:],
                                     start=True, stop=True)
                    rb = pa.tile([HD, QC], F32, tag="rb", bufs=2, name="rb")
                    nc.vector.reciprocal_approx_fast(rb[:], ps_bc[:])
                    aTn = pa.tile([HD, QC], BF16, tag="aTn", bufs=2,
                                  name="aTn")
                    nc.vector.tensor_mul(aTn[:], st['aT'][:], rb[:])
                    st['aTn'] = aTn
                return emit

            def outproj_unit(bi, qc, rc4, st):
                def emit():
                    rc = (bi * t + qc * QC) // 128 + rc4
                    ysb = py.tile([128, d], BF16, tag="ysb", name="ysb")
                    for n2 in range(2):
                        ps_y = pps.tile([128, 512], F32, tag="sh", bufs=2,
                                        name="ps_y")
                        nc.tensor.matmul(
                            ps_y[:],
                            st['aTn'][:, rc4 * 128:(rc4 + 1) * 128],
                            wo[:, n2 * 512:(n2 + 1) * 512],
                            start=True, stop=True)
                        nc.vector.tensor_copy(
                            ysb[:, n2 * 512:(n2 + 1) * 512], ps_y[:])
                    nc.sync.dma_start(y_d[rc * 128:(rc + 1) * 128, :], ysb[:])
                return emit

            # ---------- build the global schedule ----------
            attn_segs = [[], [], [], []]
            pending_early = None
            pending_late = []
            for bi in range(b):
                for qc in range(nqc):
                    seg = attn_segs[2 * bi + qc // 2]
                    st = {'pts': {}}
                    kpq = rpq * (qc + 1)
                    units = [iter_unit(bi, qc, kc, st) for kc in range(kpq)]
                    pre = [pending_early] if pending_early else []
                    seg += pre + units[:2] + weave(units[2:], pending_late)
                    pending_early = tail_early_unit(bi, qc, st)
                    pending_late = ([tail_bc_unit(bi, qc, st)]
                                    + [outproj_unit(bi, qc, r, st)
                                       for r in range(4)])
            attn_segs[3] += [pending_early] + pending_late

            sched = ([load_unit(0), load_unit(1)] + proj_units(0)
                     + weave(attn_segs[0], [load_unit(2)] + proj_units(1))
                     + weave(attn_segs[1], [load_unit(3)] + proj_units(2))
                     + weave(attn_segs[2], proj_units(3))
                     + attn_segs[3])
            for u in sched:
                u()

    nc.compile()
    return nc


def make_in_maps(x, w_qkv, w_out, b=B, t=T, d=D_MODEL):
    rows = b * t
    xr = np.asarray(x, dtype=np.float32).reshape(rows, d)
    xT = np.ascontiguousarray(xr.T).astype(NP_BF16)
    wq = np.asarray(w_qkv[:, 0:d]).reshape(d, N_HEADS, DH)
    wk = np.asarray(w_qkv[:, d:2 * d]).reshape(d, N_HEADS, DH)
    wvf = np.asarray(w_qkv[:, 2 * d:3 * d]).reshape(d, N_HEADS, DH)
    in_maps = []
    for c in range(N_CORES):
        h0, h1 = HPC * c, HPC * c + HPC
        wqkv_c = np.concatenate(
            [wq[:, h0:h1].reshape(d, HD), wk[:, h0:h1].reshape(d, HD),
             wvf[:, h0:h1].reshape(d, HD)], axis=1).astype(NP_BF16)
        wo_c = np.ascontiguousarray(w_out[h0 * DH:h1 * DH, :]).astype(NP_BF16)
        in_maps.append({"xT": xT, "wqkv": wqkv_c, "wo": wo_c})
    return in_maps


_PROGRAM_CACHE = {}


def _get_program():
    if "nc" not in _PROGRAM_CACHE:
        _PROGRAM_CACHE["nc"] = build_program()
    return _PROGRAM_CACHE["nc"]


def run(x, w_qkv, w_out, trace=False, tmpdir=None):
    nc = _get_program()
    in_maps = make_in_maps(x, w_qkv, w_out)
    res = run_bass_kernel_spmd(nc, in_maps, list(range(N_CORES)), trace=trace,
                               tmpdir=tmpdir)
    parts = np.stack([np.asarray(res.results[c]["y"], dtype=np.float32)
                      for c in range(N_CORES)])
    y = parts.sum(axis=0).reshape(B, T, D_MODEL)
    return y, res


def kernel(x, w_qkv, w_out):
    y, _ = run(x, w_qkv, w_out)
    return y
